# revision 1
# baseline (speedup 1.0000x reference)
"""GPT layer (B=2, S=2048, D=768, H=12, DK=64, HID=3072, causal) on 8 TRN2 cores.

Sharding: cores 0-3 handle batch 0, cores 4-7 batch 1. Within a 4-core group:
tensor-parallel attention over heads (3 heads/core); the W_o partial product is
ReduceScattered in two row-chunks (pipelined against attention / MLP); each
core then runs LN2 + full-width MLP on its own 512 rows (2x 256-row pieces).

All matmuls run in bf16 (fp32 PSUM accumulation). LayerNorm1's mean
subtraction is folded into the QKV projections via an extended contraction
(Q = rstd * (x@Wq - mu*colsum(Wq))), so the transposed activations x^T are
uploaded pre-transposed from the host and never normalized on-device.
Softmax skips max-subtraction (scores are O(1) by construction); its
denominator comes from a ones-column appended to V; the causal mask is a
multiplicative 0/1 bf16 mask applied after exp.
"""

import math
import os
from contextlib import ExitStack

import numpy as np
import ml_dtypes

import concourse.bass as bass
import concourse.tile as tile
from concourse import bacc, mybir
from concourse.bass_utils import run_bass_kernel_spmd
from concourse.masks import make_identity


F32 = mybir.dt.float32
BF16 = mybir.dt.bfloat16
AF = mybir.ActivationFunctionType
ALU = mybir.AluOpType
NPBF16 = ml_dtypes.bfloat16

B, S, D, H, DK, HID = 2, 2048, 768, 12, 64, 3072
EPS = 1e-5
G = 4            # cores per batch group
HG = H // G      # heads per core (3)
R = S // G       # rows per core (512)
NT = S // 128    # seq tiles (16)
DT = D // 128    # d tiles (6)
HT = HID // 128  # hid tiles (24)
CH = 4           # query chunks of 512
LEAD = 2         # score->AV software pipeline depth

_cache = {}


def _build():
    if "nc" in _cache:
        return _cache["nc"]
    nc = bacc.Bacc("TRN2", target_bir_lowering=False, num_devices=8)

    xrows_d = nc.dram_tensor("xrows", [S, D], BF16, kind="ExternalInput")
    xr_d = nc.dram_tensor("xr", [R, D], BF16, kind="ExternalInput")
    xT_d = nc.dram_tensor("xT", [128, DT * S], BF16, kind="ExternalInput")
    wqk_d = nc.dram_tensor("wqk", [128, DT * HG * 128], BF16, kind="ExternalInput")
    wv_d = nc.dram_tensor("wv", [128, DT * HG * 64], BF16, kind="ExternalInput")
    csqk_d = nc.dram_tensor("csqk", [1, HG * 128], BF16, kind="ExternalInput")
    csv_d = nc.dram_tensor("csv", [1, HG * 64], BF16, kind="ExternalInput")
    wo3_d = nc.dram_tensor("wo3", [64, HG * D], BF16, kind="ExternalInput")
    w1_d = nc.dram_tensor("w1", [128, HT * DT * 128], BF16, kind="ExternalInput")
    b1_d = nc.dram_tensor("b1r", [128, HT], F32, kind="ExternalInput")
    w2_d = nc.dram_tensor("w2", [HID, D], BF16, kind="ExternalInput")
    b2_d = nc.dram_tensor("b2r", [1, D], BF16, kind="ExternalInput")
    mask_d = nc.dram_tensor("mask", [128, 4 * 512], BF16, kind="ExternalInput")
    out_d = nc.dram_tensor("out", [R, D], F32, kind="ExternalOutput")

    with tile.TileContext(nc) as tc, ExitStack() as top:
        consts = top.enter_context(tc.tile_pool(name="consts", bufs=1))
        dram = top.enter_context(tc.tile_pool(name="dram", bufs=1, space="DRAM"))

        ident = consts.tile([128, 128], F32)
        make_identity(nc, ident[:])
        ident_bf = consts.tile([128, 128], BF16)
        make_identity(nc, ident_bf[:])
        ones_bf = consts.tile([1, 128], BF16)
        nc.vector.memset(ones_bf[:], 1.0)
        eps_sb = consts.tile([128, 1], F32)
        nc.vector.memset(eps_sb[:], EPS)
        mask_sb = consts.tile([128, 4, 512], BF16)
        nc.sync.dma_start(mask_sb[:], mask_d[:].rearrange("p (d q) -> p d q", q=512))
        wqk_sb = consts.tile([128, DT, HG, 128], BF16)
        nc.sync.dma_start(
            wqk_sb[:], wqk_d[:].rearrange("p (t h n) -> p t h n", t=DT, h=HG)
        )
        wv_sb = consts.tile([128, DT, HG, 64], BF16)
        nc.sync.dma_start(
            wv_sb[:], wv_d[:].rearrange("p (t h n) -> p t h n", t=DT, h=HG)
        )
        csqk_sb = consts.tile([1, HG, 128], BF16)
        nc.sync.dma_start(csqk_sb[:], csqk_d[:].rearrange("p (h n) -> p h n", h=HG))
        csv_sb = consts.tile([1, HG, 64], BF16)
        nc.sync.dma_start(csv_sb[:], csv_d[:].rearrange("p (h n) -> p h n", h=HG))
        wo3_sb = consts.tile([64, HG, D], BF16)
        nc.sync.dma_start(wo3_sb[:], wo3_d[:].rearrange("p (h n) -> p h n", h=HG))
        b1_sb = consts.tile([128, HT], F32)
        nc.sync.dma_start(b1_sb[:], b1_d[:])
        b2bc = consts.tile([128, D], BF16)
        nc.sync.dma_start(
            b2bc[:],
            bass.AP(tensor=b2_d[:].tensor, offset=b2_d[:].offset, ap=[[0, 128], [1, D]]),
        )

        party1 = dram.tile([2 * R, D], BF16)
        party2 = dram.tile([2 * R, D], BF16)
        rs1 = dram.tile([R // 2, D], BF16)
        rs2 = dram.tile([R // 2, D], BF16)

        with ExitStack() as attn_scope:
            apool = attn_scope.enter_context(tc.tile_pool(name="apool", bufs=1))
            stats = attn_scope.enter_context(tc.tile_pool(name="stats", bufs=8))
            scratch = attn_scope.enter_context(tc.tile_pool(name="scratch", bufs=3))
            epool = attn_scope.enter_context(tc.tile_pool(name="epool", bufs=4))
            ps_sc = attn_scope.enter_context(
                tc.tile_pool(name="ps_sc", bufs=1, space="PSUM")
            )
            ps_o = attn_scope.enter_context(
                tc.tile_pool(name="ps_o", bufs=1, space="PSUM")
            )
            ps_qkv = attn_scope.enter_context(
                tc.tile_pool(name="ps_qkv", bufs=1, space="PSUM")
            )
            ps_w = attn_scope.enter_context(
                tc.tile_pool(name="ps_w", bufs=1, space="PSUM")
            )

            xT_sb = apool.tile([128, DT, S], BF16)
            for dt in range(DT):
                nc.sync.dma_start(
                    xT_sb[:, dt, 0:512], xT_d[:, dt * S: dt * S + 512]
                )

            QT = apool.tile([64, HG, S], BF16)
            KT = apool.tile([64, HG, S], BF16)
            Vg = apool.tile([128, NT, HG, DK + 1], BF16)
            nc.vector.memset(Vg[:, :, :, DK:DK + 1], 1.0)
            OT = apool.tile([64, HG, S], BF16)
            rstd_bc = apool.tile([128, S], F32)
            muT = apool.tile([1, S], BF16)   # -mean, transposed to a row
            rsT = apool.tile([1, S], BF16)   # rstd, transposed to a row
            negmean_all = apool.tile([128, NT], F32)
            rstd_all = apool.tile([128, NT], F32)

            # ---- LN1 statistics from row-layout x (all upfront) ----
            for st in range(NT):
                xt = scratch.tile([128, D], BF16, tag="xin")
                nc.sync.dma_start(xt[:], xrows_d[st * 128:(st + 1) * 128, :])
                bn6 = stats.tile([128, 3, 6], F32, tag="bn6")
                for sg in range(3):
                    nc.vector.bn_stats(bn6[:, sg, :], xt[:, sg * 256:(sg + 1) * 256])
                mv = stats.tile([128, 2], F32, tag="mv")
                nc.vector.bn_aggr(mv[:], bn6[:])
                nc.vector.tensor_scalar_mul(
                    negmean_all[:, st:st + 1], mv[:, 0:1], -1.0
                )
                std = stats.tile([128, 1], F32, tag="std")
                nc.scalar.activation(std[:], mv[:, 1:2], AF.Sqrt, bias=eps_sb[:])
                nc.vector.reciprocal(rstd_all[:, st:st + 1], std[:])

            # remaining x^T chunks (chunk 0 was queued before the stats DMAs)
            for c in range(1, CH):
                for dt in range(DT):
                    nc.sync.dma_start(
                        xT_sb[:, dt, c * 512:(c + 1) * 512],
                        xT_d[:, dt * S + c * 512: dt * S + (c + 1) * 512],
                    )

            # transpose stats to single-partition rows (legal matmul operands),
            # then broadcast rstd along partitions via K=1 matmuls
            for c in range(CH):
                ptm = ps_w.tile([128, 512], F32, tag="pw", bufs=1)
                for j in range(4):
                    nc.tensor.transpose(
                        ptm[0:1, j * 128:(j + 1) * 128],
                        negmean_all[:, 4 * c + j:4 * c + j + 1], ident[:],
                    )
                nc.vector.tensor_copy(muT[:, c * 512:(c + 1) * 512], ptm[0:1, :])
                ptr = ps_w.tile([128, 512], F32, tag="pw", bufs=1)
                for j in range(4):
                    nc.tensor.transpose(
                        ptr[0:1, j * 128:(j + 1) * 128],
                        rstd_all[:, 4 * c + j:4 * c + j + 1], ident[:],
                    )
                nc.vector.tensor_copy(rsT[:, c * 512:(c + 1) * 512], ptr[0:1, :])
                pbx = ps_w.tile([128, 512], F32, tag="pw", bufs=1)
                for j in range(4):
                    nc.tensor.matmul(
                        pbx[:, j * 128:(j + 1) * 128], ones_bf[:],
                        rsT[:, (4 * c + j) * 128:(4 * c + j + 1) * 128],
                        start=True, stop=True, skip_group_check=True,
                    )
                nc.vector.tensor_copy(rstd_bc[:, c * 512:(c + 1) * 512], pbx[:])

            # ---- attention, chunk-pipelined over query blocks of 512 ----
            for c in range(CH):
                cs = c * 512
                # QK projections for this chunk (packed Q|K per head)
                for h in range(HG):
                    pqk = ps_qkv.tile([128, 512], F32, tag="pqk", bufs=1)
                    for dt in range(DT):
                        nc.tensor.matmul(
                            pqk[:], wqk_sb[:, dt, h, :], xT_sb[:, dt, cs:cs + 512],
                            start=(dt == 0), stop=False,
                        )
                    for j in range(4):
                        nc.tensor.matmul(
                            pqk[:, j * 128:(j + 1) * 128], csqk_sb[:, h, :],
                            muT[:, (4 * c + j) * 128:(4 * c + j + 1) * 128],
                            start=False, stop=True, skip_group_check=True,
                        )
                    nc.vector.tensor_mul(
                        QT[:, h, cs:cs + 512], pqk[0:64, :],
                        rstd_bc[0:64, cs:cs + 512],
                    )
                    nc.vector.tensor_mul(
                        KT[:, h, cs:cs + 512], pqk[64:128, :],
                        rstd_bc[64:128, cs:cs + 512],
                    )
                # V for the 4 key tiles of this chunk
                for j in range(4):
                    st = 4 * c + j
                    pv = ps_qkv.tile([128, HG * 64], F32, tag="pv", bufs=1)
                    for dt in range(DT):
                        nc.tensor.matmul(
                            pv[:], xT_sb[:, dt, st * 128:(st + 1) * 128],
                            wv_sb[:, dt, :, :],
                            start=(dt == 0), stop=False,
                        )
                    nc.tensor.matmul(
                        pv[:], muT[:, st * 128:(st + 1) * 128], csv_sb[:, :, :],
                        start=False, stop=True,
                    )
                    nc.vector.tensor_scalar(
                        Vg[:, st, :, 0:DK],
                        pv[:].rearrange("p (h n) -> p h n", h=HG),
                        rstd_all[:, st:st + 1], None, op0=ALU.mult,
                    )

                # scores -> exp -> (mask) -> A@V, software-pipelined
                ntl = 4 * (c + 1)
                for h in range(HG):
                    po = ps_o.tile([DK + 1, 512], F32, tag="po", bufs=2)
                    es = {}
                    q0s = {}
                    for i in range(ntl + LEAD):
                        if i < ntl:
                            t = i
                            dd = t - 4 * c
                            # diagonal-band tiles: queries < dd*128 are fully
                            # masked, so compute only the live query range
                            q0 = dd * 128 if dd > 0 else 0
                            q0s[t] = q0
                            psc = ps_sc.tile([128, 512], F32, tag="psc", bufs=3)
                            nc.tensor.matmul(
                                psc[:, q0:512], KT[:, h, t * 128:(t + 1) * 128],
                                QT[:, h, cs + q0:cs + 512], start=True, stop=True,
                            )
                            if dd >= 0:
                                e_r = epool.tile([128, 512], BF16, tag="e", bufs=4)
                                nc.scalar.activation(
                                    e_r[:, q0:512], psc[:, q0:512], AF.Exp
                                )
                                e_t = epool.tile([128, 512], BF16, tag="em", bufs=4)
                                nc.vector.tensor_mul(
                                    e_t[:, q0:512], e_r[:, q0:512],
                                    mask_sb[:, dd, q0:512],
                                )
                            else:
                                e_t = epool.tile([128, 512], BF16, tag="e", bufs=4)
                                nc.scalar.activation(e_t[:], psc[:], AF.Exp)
                            es[t] = e_t
                        j = i - LEAD
                        if j >= 0:
                            jq0 = q0s.pop(j)
                            nc.tensor.matmul(
                                po[:, jq0:512], Vg[:, j, h, :],
                                es.pop(j)[:, jq0:512],
                                start=(j == 0), stop=(j == ntl - 1),
                            )
                    # normalize: OT = po[:DK] * broadcast(1/po[DK])
                    den_s = stats.tile([1, 512], F32, tag="den")
                    nc.vector.tensor_copy(den_s[:], po[DK:DK + 1, :])
                    rec_f = stats.tile([1, 512], F32, tag="rec_f")
                    nc.vector.reciprocal_approx_fast(rec_f[:], den_s[:])
                    rec = stats.tile([1, 512], BF16, tag="rec")
                    nc.vector.tensor_copy(rec[:], rec_f[:])
                    rb = epool.tile([64, 512], BF16, tag="rb", bufs=2)
                    nc.gpsimd.partition_broadcast(rb[:], rec[:])
                    nc.vector.tensor_mul(OT[:, h, cs:cs + 512], po[0:DK, :], rb[:])

                # W_o partial for this chunk's 4 row tiles
                for rt in range(4 * c, 4 * c + 4):
                    party_d = party1 if rt < 8 else party2
                    row0 = (rt % 8) * 128
                    for n0, nw in ((0, 512), (512, 256)):
                        pw = ps_w.tile([128, 512], F32, tag="pw", bufs=1)
                        for hh in range(HG):
                            nc.tensor.matmul(
                                pw[:, 0:nw], OT[:, hh, rt * 128:(rt + 1) * 128],
                                wo3_sb[:, hh, n0:n0 + nw],
                                start=(hh == 0), stop=(hh == HG - 1),
                            )
                        prow = scratch.tile([128, 512], BF16, tag="prow")
                        nc.vector.tensor_copy(prow[:, 0:nw], pw[:, 0:nw])
                        nc.sync.dma_start(
                            party_d[row0:row0 + 128, n0:n0 + nw], prow[:, 0:nw]
                        )

                if c == 1:
                    nc.gpsimd.collective_compute(
                        "ReduceScatter", ALU.add,
                        replica_groups=[[0, 1, 2, 3], [4, 5, 6, 7]],
                        ins=[party1[:].opt()], outs=[rs1[:].opt()],
                    )

        nc.gpsimd.collective_compute(
            "ReduceScatter", ALU.add,
            replica_groups=[[0, 1, 2, 3], [4, 5, 6, 7]],
            ins=[party2[:].opt()], outs=[rs2[:].opt()],
        )

        # ---- LN2 + MLP over two 256-row halves ----
        with ExitStack() as mlp_scope:
            mpool = mlp_scope.enter_context(tc.tile_pool(name="mpool", bufs=1))
            mstats = mlp_scope.enter_context(tc.tile_pool(name="mstats", bufs=8))
            mscratch = mlp_scope.enter_context(tc.tile_pool(name="mscratch", bufs=3))
            w1pool = mlp_scope.enter_context(tc.tile_pool(name="w1pool", bufs=3))
            w2pool = mlp_scope.enter_context(tc.tile_pool(name="w2pool", bufs=3))
            ps_t2 = mlp_scope.enter_context(
                tc.tile_pool(name="ps_t2", bufs=1, space="PSUM")
            )
            ps_f1 = mlp_scope.enter_context(
                tc.tile_pool(name="ps_f1", bufs=1, space="PSUM")
            )
            ps_f2 = mlp_scope.enter_context(
                tc.tile_pool(name="ps_f2", bufs=1, space="PSUM")
            )

            y_sb = mpool.tile([128, 4, D], BF16)
            h2T = mpool.tile([128, DT, R], BF16)
            gT = mpool.tile([128, HT, R], BF16)

            for hf in range(2):
                rs_d = rs1 if hf == 0 else rs2
                for m in range(2):
                    rl = hf * 256 + m * 128  # local row offset
                    rs_t = mscratch.tile([128, D], BF16, tag="rst")
                    nc.sync.dma_start(rs_t[:], rs_d[m * 128:(m + 1) * 128, :])
                    xr_t = mscratch.tile([128, D], BF16, tag="xrt")
                    nc.sync.dma_start(xr_t[:], xr_d[rl:rl + 128, :])
                    nc.vector.tensor_add(y_sb[:, hf * 2 + m, :], rs_t[:], xr_t[:])
                    bn6 = mstats.tile([128, 3, 6], F32, tag="bn6")
                    for sg in range(3):
                        nc.vector.bn_stats(
                            bn6[:, sg, :],
                            y_sb[:, hf * 2 + m, sg * 256:(sg + 1) * 256],
                        )
                    mv = mstats.tile([128, 2], F32, tag="mv")
                    nc.vector.bn_aggr(mv[:], bn6[:])
                    std = mstats.tile([128, 1], F32, tag="std")
                    nc.scalar.activation(std[:], mv[:, 1:2], AF.Sqrt, bias=eps_sb[:])
                    rstd = mstats.tile([128, 1], F32, tag="rstd")
                    nc.vector.reciprocal(rstd[:], std[:])
                    h2_t = mscratch.tile([128, D], BF16, tag="h2row")
                    nc.vector.tensor_scalar(
                        h2_t[:], y_sb[:, hf * 2 + m, :], mv[:, 0:1], rstd[:],
                        op0=ALU.subtract, op1=ALU.mult,
                    )
                    for half in range(2):
                        pt = ps_t2.tile([128, 384], BF16, tag="ptr", bufs=2)
                        for k in range(3):
                            dt = half * 3 + k
                            nc.tensor.transpose(
                                pt[:, k * 128:(k + 1) * 128],
                                h2_t[:, dt * 128:(dt + 1) * 128], ident_bf[:],
                            )
                        nc.vector.tensor_copy(
                            h2T[:, half * 3:half * 3 + 3, rl:rl + 128], pt[:]
                        )

                # fc1 + gelu for this half
                for hc in range(HT):
                    w1c = w1pool.tile([128, DT, 128], BF16, tag="w1c")
                    nc.sync.dma_start(
                        w1c[:],
                        w1_d[:, hc * DT * 128:(hc + 1) * DT * 128]
                        .rearrange("p (t n) -> p t n", t=DT),
                    )
                    pf = ps_f1.tile([128, 256], F32, tag="pf", bufs=2)
                    for dt in range(DT):
                        nc.tensor.matmul(
                            pf[:], w1c[:, dt, :],
                            h2T[:, dt, hf * 256:(hf + 1) * 256],
                            start=(dt == 0), stop=(dt == DT - 1),
                        )
                    nc.scalar.activation(
                        gT[:, hc, hf * 256:(hf + 1) * 256], pf[:], AF.Gelu,
                        bias=b1_sb[:, hc:hc + 1],
                    )

                # fc2 for this half (W2 streamed)
                pacc = {}
                for m in range(2):
                    for n0, nw in ((0, 512), (512, 256)):
                        pacc[(m, n0)] = ps_f2.tile(
                            [128, nw], F32, tag=f"pf2_{m}_{n0}", bufs=1,
                            name=f"pf2_{m}_{n0}",
                        )
                for t in range(HT):
                    w2t = w2pool.tile([128, D], BF16, tag="w2t")
                    nc.sync.dma_start(w2t[:], w2_d[t * 128:(t + 1) * 128, :])
                    for m in range(2):
                        rl = hf * 256 + m * 128
                        for n0, nw in ((0, 512), (512, 256)):
                            nc.tensor.matmul(
                                pacc[(m, n0)], gT[:, t, rl:rl + 128],
                                w2t[:, n0:n0 + nw],
                                start=(t == 0), stop=(t == HT - 1),
                            )
                for m in range(2):
                    yb = mscratch.tile([128, D], BF16, tag="yb")
                    nc.vector.tensor_add(yb[:], y_sb[:, hf * 2 + m, :], b2bc[:])
                    o_t = mscratch.tile([128, D], F32, tag="ot")
                    for n0, nw in ((0, 512), (512, 256)):
                        nc.vector.tensor_add(
                            o_t[:, n0:n0 + nw], pacc[(m, n0)], yb[:, n0:n0 + nw]
                        )
                    nc.sync.dma_start(
                        out_d[(hf * 2 + m) * 128:(hf * 2 + m + 1) * 128, :], o_t[:]
                    )

    nc.finalize()
    _cache["nc"] = nc
    return nc


def _mask_np():
    # mask[p, dd*512 + q] = 1 where key (dd*128 + p) <= query q, else 0
    m = np.zeros((128, 4 * 512), dtype=NPBF16)
    p = np.arange(128)[:, None]
    q = np.arange(512)[None, :]
    for dd in range(4):
        m[:, dd * 512:(dd + 1) * 512] = (dd * 128 + p <= q).astype(NPBF16)
    return m


def kernel(x, Wq, Wk, Wv, Wo, W1, b1, W2, b2, g_ln1, b_ln1, g_ln2, b_ln2):
    x = np.asarray(x, dtype=np.float32)
    Wq = np.asarray(Wq, dtype=np.float32)
    Wk = np.asarray(Wk, dtype=np.float32)
    Wv = np.asarray(Wv, dtype=np.float32)
    Wo = np.asarray(Wo, dtype=np.float32)
    W1 = np.asarray(W1, dtype=np.float32)
    b1 = np.asarray(b1, dtype=np.float32)
    W2 = np.asarray(W2, dtype=np.float32)
    b2 = np.asarray(b2, dtype=np.float32)
    g_ln1 = np.asarray(g_ln1, dtype=np.float32)
    b_ln1 = np.asarray(b_ln1, dtype=np.float32)
    g_ln2 = np.asarray(g_ln2, dtype=np.float32)
    b_ln2 = np.asarray(b_ln2, dtype=np.float32)
    assert not np.any(b_ln1), "nonzero b_ln1 not supported by this kernel"

    nc = _build()
    mask = _mask_np()
    scale = 1.0 / math.sqrt(DK)

    # LN2 gain folds into W1 (exactly); LN2 bias folds into the fc1 bias.
    W1_eff = g_ln2[:, None] * W1
    b1_eff = b1 + b_ln2 @ W1
    w1_r = np.ascontiguousarray(
        W1_eff.reshape(DT, 128, HT, 128).transpose(1, 2, 0, 3).reshape(128, -1)
    ).astype(NPBF16)
    b1r = np.ascontiguousarray(b1_eff.reshape(HT, 128).T).astype(np.float32)
    w2_bf = W2.astype(NPBF16)
    b2r = b2.reshape(1, D).astype(NPBF16)

    in_maps = []
    for core in range(8):
        b, r = core // G, core % G
        hsl = slice(HG * r, HG * (r + 1))
        # [D, HG, 64] with LN1 gain folded in; Q side also folds 1/sqrt(dk)
        wq3 = (Wq[hsl] * g_ln1[None, :, None]).transpose(1, 0, 2) * scale
        wk3 = (Wk[hsl] * g_ln1[None, :, None]).transpose(1, 0, 2)
        wv3 = (Wv[hsl] * g_ln1[None, :, None]).transpose(1, 0, 2)
        wqk = np.concatenate([wq3, wk3], axis=2)  # [D, HG, 128]
        wqk_r = np.ascontiguousarray(
            wqk.reshape(DT, 128, HG, 128).transpose(1, 0, 2, 3).reshape(128, -1)
        ).astype(NPBF16)
        wv_r = np.ascontiguousarray(
            wv3.reshape(DT, 128, HG, 64).transpose(1, 0, 2, 3).reshape(128, -1)
        ).astype(NPBF16)
        csqk = wqk.sum(axis=0).reshape(1, -1).astype(NPBF16)
        csv = wv3.sum(axis=0).reshape(1, -1).astype(NPBF16)
        wo_c = Wo[HG * DK * r:HG * DK * (r + 1), :]
        wo3 = np.ascontiguousarray(
            wo_c.reshape(HG, DK, D).transpose(1, 0, 2).reshape(DK, HG * D)
        ).astype(NPBF16)
        xb = x[b].astype(NPBF16)
        xT_r = np.ascontiguousarray(
            xb.T.reshape(DT, 128, S).transpose(1, 0, 2).reshape(128, -1)
        )
        # core's MLP rows: 256 from each ReduceScatter chunk
        rows1 = slice(r * 256, (r + 1) * 256)
        rows2 = slice(1024 + r * 256, 1024 + (r + 1) * 256)
        xr = np.concatenate([xb[rows1], xb[rows2]], axis=0)
        in_maps.append({
            "xrows": np.ascontiguousarray(xb),
            "xr": np.ascontiguousarray(xr),
            "xT": xT_r,
            "wqk": wqk_r, "wv": wv_r,
            "csqk": np.ascontiguousarray(csqk), "csv": csv,
            "wo3": wo3,
            "w1": w1_r, "b1r": b1r, "w2": w2_bf, "b2r": b2r,
            "mask": mask,
        })

    trace = bool(int(os.environ.get("BENCH_TRACE", "0")))
    res = run_bass_kernel_spmd(nc, in_maps, core_ids=list(range(8)), trace=trace)
    _cache["last_results"] = res

    out = np.empty((B, S, D), dtype=np.float32)
    for core in range(8):
        b, r = core // G, core % G
        o = res.results[core]["out"]
        out[b, r * 256:(r + 1) * 256, :] = o[0:256]
        out[b, 1024 + r * 256:1024 + (r + 1) * 256, :] = o[256:512]
    return out

